# revision 31
# baseline (speedup 1.0000x reference)
"""DNDT (deep neural decision tree) forward kernel for 8 Trainium2 NeuronCores.

Math (per batch row b of 16384):
  h[f,j]   = (x[b,f] * W[j] + bias[f,j]) / t,  W = [1..4], bias = cumsum([0,-sorted_cuts])
  bins     = softmax_j(h)                       # [6, 4]
  leaf     = kron(bins[0], ..., bins[5])        # [4096]
  out[b]   = leaf @ leaf_score                  # [10]

Device algorithm (pure data parallel, 2048 rows/core, batch-major layout
[128 partitions x 16 rows-per-partition x ...]):
  * softmax shift uses the analytic bound g(x) = (x + 3*relu(x))/t instead of a
    max-reduce (softmax is shift invariant; exp args stay <= 0), and the
    bias/t term is pulled OUT of the exp entirely: exp(bias/t) kron-factors
    are folded into the score matrix on the host (safe in bf16: A <= 1
    bounds every term by its column const).  Device exponent is just
    h' = x*(W[j]-1)/t - 3/t*relu(x), built as one relu-scale plus three
    fused (x*j/t + h0) scalar_tensor_tensor ops.
  * unnormalized bins E = exp(h'); leaf never materialized:
    A = bins0*bins1*bins2*bins3 kron (256, bf16), p45 = bins4*bins5 kron (16).
  * normalizer via an extra 16-col block appended to the score matrix
    (the folded consts where the plain version has all-ones), so the same
    mul+reduce that contracts the class blocks also produces the softmax
    denominator.  out = O[:, :10] * recip(O[:, 10]); no Z/zp/zr chain.
  * matmul path in bf16 (A, at2, S2): full-rate PE with half-size LDWEIGHTS;
    rel err ~2e-3 (bf16 has the range for the e-35 normalizer tails, fp16
    does not). Each PSUM matmul slot padded to 256 f32 so its 176 cols
    never cross a 2KB PSUM bank (bank-crossing corrupts accumulation).
  * back half per 2-slot half-group: DVE multiplies D = C*p45 straight
    from PSUM and reduces O; ACT only does exp and the at2 staging copies
    (offloading D via ACT-copy + Pool lost to SBUF port contention).
  * junk matmuls on x warm the PE HAM clock gate while the DVE front runs.
  * two input DMAs total (x fp32 on SP, s2+ident bf16 on ACT): each
    DIRECT2D issue costs ~650ns on a sequencer.
"""

import numpy as np

import concourse.bass as bass
import concourse.tile as tile
from concourse import bacc, mybir
from concourse.bass_utils import run_bass_kernel_spmd

N_CORES = 8
B = 16384
BC = B // N_CORES          # rows per core = 2048
P = 128                    # partitions
M = BC // P                # rows per partition = 16
NCHUNK = 2                 # pipeline chunks
CHM = M // NCHUNK          # rows per partition per chunk = 8
QS = 4                     # row-slots per transpose/matmul group
F32 = mybir.dt.float32
BF16 = mybir.dt.bfloat16
N_WARM = 8                 # junk matmuls to warm the PE clock gate
NBLK = 11                  # 10 class blocks + 1 all-ones normalizer block
WID = NBLK * 16            # 176


def _build_nc(neg3invt):
    nc = bacc.Bacc("TRN2", target_bir_lowering=False, debug=False,
                   num_devices=N_CORES)
    xd = nc.dram_tensor("xc", [P, M * 6], F32, kind="ExternalInput")
    sid = nc.dram_tensor("si", [P, 2 * WID + P], BF16, kind="ExternalInput")
    od = nc.dram_tensor("o", [P, M * 10], F32, kind="ExternalOutput")

    with tile.TileContext(nc) as tc:
        with tc.tile_pool(name="consts", bufs=1) as consts, \
             tc.tile_pool(name="work", bufs=3) as work, \
             tc.tile_pool(name="atp", bufs=3) as atp, \
             tc.tile_pool(name="ps_t", bufs=2, space="PSUM") as ps_t, \
             tc.tile_pool(name="ps_w", bufs=2, space="PSUM") as ps_w, \
             tc.tile_pool(name="ps_c", bufs=4, space="PSUM") as ps_c:
            xc_st = consts.tile([P, M * 6], F32)
            nc.sync.dma_start(out=xc_st[:], in_=xd[:])
            xv3 = xc_st[:, 0:M * 6].rearrange("p (i f) -> p i f", i=M)

            # HAM warm-up: junk matmuls on the otherwise idle PE while the
            # front (DMAs, DVE H/E/kron) runs.  fp32 (slow path) on purpose:
            # more PE-busy cycles per instruction.
            def warm_mm(n):
                for _ in range(n):
                    wps = ps_w.tile([P, 4, P], F32, tag="wp")
                    nc.tensor.matmul(wps[0:M * 6, 0, 0:M * 6],
                                     lhsT=xc_st[:], rhs=xc_st[:],
                                     start=True, stop=True)
            warm_mm(N_WARM)

            si_st = consts.tile([P, 2 * WID + P], BF16)
            nc.scalar.dma_start(out=si_st[:], in_=sid[:])
            s2_sb = si_st[:, 0:2 * WID].rearrange("p (k n) -> p k n", k=2)
            ident = si_st[:, 2 * WID:]

            invt = -neg3invt / 3.0
            for c in range(NCHUNK):
                xv = xv3[:, c * CHM:(c + 1) * CHM, :]
                # bias/t is folded into S2 host-side (safe in bf16: A <= 1
                # bounds every term by its column const, so flushed columns
                # contribute < 1e-38 to sums >= 6e-35).  The H chain is then
                # H[...,0] = -3/t*relu(x) and H[...,j] = x*j/t + H[...,0]:
                # four small fused ops instead of three 192-el broadcast ops.
                H = work.tile([P, CHM, 6, 4], F32, tag="H")
                nc.vector.tensor_scalar(out=H[:, :, :, 0], in0=xv,
                                        scalar1=0.0, scalar2=neg3invt,
                                        op0=mybir.AluOpType.max, op1=mybir.AluOpType.mult)
                for j in range(1, 4):
                    nc.vector.scalar_tensor_tensor(
                        out=H[:, :, :, j], in0=xv, scalar=float(j) * invt,
                        in1=H[:, :, :, 0],
                        op0=mybir.AluOpType.mult, op1=mybir.AluOpType.add)
                E = work.tile([P, CHM, 6, 4], F32, tag="E")
                nc.scalar.activation(E[:].rearrange("p i f j -> p (i f j)"),
                                     H[:].rearrange("p i f j -> p (i f j)"),
                                     mybir.ActivationFunctionType.Exp)

                # all three pair-krons (p01, p23, p45) in ONE op via an
                # even/odd feature split: fewer DVE instruction overheads.
                pp = work.tile([P, CHM, 3, 16], F32, tag="pp")
                E2 = E[:].rearrange("p i (g t) j -> p i g t j", t=2)
                nc.vector.tensor_mul(
                    pp[:].rearrange("p i f (a b) -> p i f a b", a=4),
                    E2[:, :, :, 0, :, None].broadcast_to((P, CHM, 3, 4, 4)),
                    E2[:, :, :, 1, None, :].broadcast_to((P, CHM, 3, 4, 4)))
                p45 = pp[:, :, 2, :]
                # one A tile per 4-slot group (deps are tile-granular), each
                # written by a single mul: same dep shape, half the op count.
                Ag = [work.tile([P, QS, 256], BF16, tag=f"A{gg}",
                                name=f"A{c}_{gg}")
                      for gg in range(CHM // QS)]
                for gg in range(CHM // QS):
                    slp = slice(gg * QS, (gg + 1) * QS)
                    nc.vector.tensor_mul(
                        Ag[gg][:].rearrange("p i (a b) -> p i a b", a=16),
                        pp[:, slp, 0, :, None].broadcast_to((P, QS, 16, 16)),
                        pp[:, slp, 1, None, :].broadcast_to((P, QS, 16, 16)))

                Ogc = None
                for g in range(CHM // QS):
                    base = g * QS
                    A = Ag[g]
                    Og = work.tile([P, QS, NBLK], F32, tag="O",
                                   name=f"Og{c}_{g}")[:]
                    cpph = []
                    for h in range(2):          # fully per-2-slot-half pipeline
                        tp = ps_t.tile([P, 4, P], BF16, tag="tp")
                        for jj in range(2):
                            for k in range(2):
                                nc.tensor.transpose(tp[:, jj * 2 + k, :],
                                                    A[:, 2 * h + jj, k * P:(k + 1) * P],
                                                    ident[:])
                        at2 = atp.tile([P, 4, P], BF16, tag="at")
                        nc.scalar.copy(out=at2[:], in_=tp[:])
                        # pad each slot to 256 f32 so a slot's 176 cols never
                        # cross a 2KB PSUM bank (matmul out must stay in-bank)
                        cpp = ps_c.tile([P, 2, 256], F32, tag="cp")
                        cpph.append(cpp)
                        for jj in range(2):
                            nc.tensor.matmul(cpp[:, jj, 0:WID], lhsT=at2[:, jj * 2, :],
                                             rhs=s2_sb[:, 0, :], start=True, stop=False)
                            nc.tensor.matmul(cpp[:, jj, 0:WID], lhsT=at2[:, jj * 2 + 1, :],
                                             rhs=s2_sb[:, 1, :], start=False, stop=True)
                    last = (c == NCHUNK - 1) and (g == CHM // QS - 1)
                    D4 = work.tile([P, QS, NBLK, 16], F32, tag="D")
                    for h2 in range(2):
                        i2 = slice(2 * h2, 2 * h2 + 2)
                        sl2 = slice(base + 2 * h2, base + 2 * h2 + 2)
                        if last and h2 == 1:
                            # very last half: DVE direct from PSUM (shortest
                            # serial chain); this group's other half runs
                            # through Pool in parallel so the two halves don't
                            # serialize ~2.4us on DVE at the pipeline tail.
                            nc.vector.tensor_mul(
                                D4[:, i2],
                                cpph[h2][:, :, 0:WID].rearrange("p i (c v) -> p i c v", c=NBLK),
                                p45[:, sl2, None, :].broadcast_to((P, 2, NBLK, 16)))
                        else:
                            # ACT stages cpp to SBUF, idle Pool does the D-mul
                            # per 2-slot half (whole-group 1.7us Pool muls
                            # starved DVE of O work for 1.8us).
                            cppsb = work.tile([P, 2, WID], F32, tag="cs")
                            # boost the staging copy above the NEXT group's
                            # at2 copies on ACT: the tile scheduler picks by
                            # readiness, and a late stage-copy delays Pool.
                            with tc.high_priority(offset=40):
                                nc.scalar.copy(out=cppsb[:],
                                               in_=cpph[h2][:, :, 0:WID])
                            nc.gpsimd.tensor_mul(
                                D4[:, i2],
                                cppsb[:].rearrange("p i (c v) -> p i c v", c=NBLK),
                                p45[:, sl2, None, :].broadcast_to((P, 2, NBLK, 16)))
                        if last:
                            # tail: reduce per half so the final O covers only
                            # the last 2 slots
                            nc.vector.tensor_reduce(Og[:, i2, :], D4[:, i2],
                                                    axis=mybir.AxisListType.X,
                                                    op=mybir.AluOpType.add)
                    if not last:
                        # one reduce per group: same dependency shape (D4 is
                        # tile-granular anyway), one less DVE op overhead
                        nc.vector.tensor_reduce(Og, D4[:],
                                                axis=mybir.AxisListType.X,
                                                op=mybir.AluOpType.add)
                    if not last and Ogc is None:
                        zr = work.tile([P, QS, 1], F32, tag="zr")
                        nc.vector.reciprocal(zr[:, :, 0], Og[:, :, 10])
                        Of = work.tile([P, QS, 10], F32, tag="Of")
                        nc.vector.tensor_mul(Of[:], Og[:, :, 0:10],
                                             zr[:].broadcast_to((P, QS, 10)))
                        row0 = c * CHM + base
                        nc.sync.dma_start(
                            out=od[:].rearrange("p (i c) -> p i c", i=M)[:, row0:row0 + QS, :],
                            in_=Of[:])
                    else:
                        # last group: finalize + DMA per 2-slot half so the
                        # terminal serial chain covers only the final half
                        # (its sibling's output is already in flight).
                        for h2 in range(2):
                            i2 = slice(2 * h2, 2 * h2 + 2)
                            zr = work.tile([P, 2, 1], F32, tag="zrl")
                            nc.vector.reciprocal(zr[:, :, 0], Og[:, i2, 10])
                            Of = work.tile([P, 2, 10], F32, tag="Ofl")
                            nc.vector.tensor_mul(Of[:], Og[:, i2, 0:10],
                                                 zr[:].broadcast_to((P, 2, 10)))
                            row0 = c * CHM + base + 2 * h2
                            nc.sync.dma_start(
                                out=od[:].rearrange("p (i c) -> p i c", i=M)[:, row0:row0 + 2, :],
                                in_=Of[:])
                if Ogc is not None:
                    # chunk 0: one finalize + one DMA for all 8 slots (not on
                    # the pipeline tail, so the cross-group wait is free and
                    # saves a recip + mul + DMA issue)
                    zr = work.tile([P, CHM, 1], F32, tag="zc")
                    nc.vector.reciprocal(zr[:, :, 0], Ogc[:, :, 10])
                    Of = work.tile([P, CHM, 10], F32, tag="Ofc")
                    nc.vector.tensor_mul(Of[:], Ogc[:, :, 0:10],
                                         zr[:].broadcast_to((P, CHM, 10)))
                    row0 = c * CHM
                    nc.sync.dma_start(
                        out=od[:].rearrange("p (i c) -> p i c", i=M)[:, row0:row0 + CHM, :],
                        in_=Of[:])
    nc.compile()
    return nc


def prep_inputs(x, cuts, leaf_score, temperature):
    """Host-side parameter prep (tiny). Returns (in_maps, invt)."""
    import ml_dtypes
    x = np.ascontiguousarray(np.asarray(x, dtype=np.float32))
    cuts = np.asarray(cuts, dtype=np.float32)
    leaf_score = np.asarray(leaf_score, dtype=np.float32)
    invt = 1.0 / float(np.asarray(temperature).reshape(-1)[0])

    sc = np.sort(cuts, axis=1)
    bias = np.cumsum(np.concatenate([np.zeros((6, 1), np.float64), -sc], axis=1,
                                    dtype=np.float64), axis=1)          # [6,4]
    ebt = np.exp(bias * invt)                                            # [6,4]
    c0123 = np.einsum('a,b,c,d->abcd', ebt[0], ebt[1], ebt[2],
                      ebt[3]).reshape(256)
    c45 = np.einsum('a,b->ab', ebt[4], ebt[5]).reshape(16)
    xs = x.reshape(N_CORES, P, M * 6)

    s2 = np.zeros((256, WID), np.float64)
    s2[:, :160] = leaf_score.reshape(256, 16, 10).transpose(0, 2, 1).reshape(256, 160)
    s2[:, 160:] = 1.0
    s2 = s2 * c0123[:, None] * np.tile(c45, NBLK)[None, :]
    s2 = s2.reshape(2, P, WID)
    si = np.concatenate([s2[0], s2[1], np.eye(P)], axis=1)
    si = np.ascontiguousarray(si.astype(ml_dtypes.bfloat16))

    in_maps = [{"xc": np.ascontiguousarray(xs[i]), "si": si}
               for i in range(N_CORES)]
    return in_maps, invt


_CACHE = {}


def kernel(x, cuts, leaf_score, temperature):
    in_maps, invt = prep_inputs(x, cuts, leaf_score, temperature)
    key = ("nc", float(invt))
    if key not in _CACHE:
        _CACHE[key] = _build_nc(-3.0 * invt)
        _CACHE["nc"] = _CACHE[key]
    nc = _CACHE[key]
    res = run_bass_kernel_spmd(nc, in_maps, list(range(N_CORES))).results
    out = np.concatenate([r["o"].reshape(BC, 10) for r in res], axis=0)
    return out.astype(np.float32)
